# revision 18
# baseline (speedup 1.0000x reference)
"""Causal multi-head self-attention (RoPE) Trainium2 Bass kernel.

Problem: x[4,2048,1024] f32, Wq/Wk/Wv/Wo[1024,1024], token_positions[2048].
  q,k,v = x@W.T per head (16 heads, dk=64); RoPE(q,k); causal softmax(q k^T/8) @ v;
  concat heads @ Wo.T.

Sharding (8 cores): core c -> batch b=c//2, head-group hg=c%2 (8 heads each).
Each core computes a partial output (its 8 heads' contribution through Wo);
host sums the two partials per batch.

On-chip layouts (per core):
  xT      [128, 8, 2048] bf16  : x[b].T chunked over d_model (DMA per chunk)
  qT/kT   [128, 512] bf16 per (pair, s-tile): rows = rope-permuted dims of a
          head pair: [A-even(0:32) A-odd(32:64) B-even(64:96) B-odd(96:128)]
  v       [128, 512] bf16 per s-chunk (8 heads x 64)
  scoresT [128, 2, 512] psum; the two heads' K=64 matmuls are issued
          adjacently with explicit tile_position=(64h,0) so they run
          concurrently in separate PE row groups; exp on ACT -> attnT bf16;
          causal mask via gpsimd affine_select on diagonal blocks; AV pair
          col-tiled into one psum bank (64 v dims + 64 ones rows for the
          denominator); reciprocal+mul normalizes.

Schedule: phase_b is software-pipelined (scores of chunk i+1 issue before
AV of chunk i, hiding the EXP); Wo projections for tiles 0..2 are deferred
until after phase_a(3) so their matmuls fill the tensor-idle gaps of the
ACT-bound phase_b(3).
"""

from contextlib import ExitStack

import numpy as np
import ml_dtypes

import concourse.bass as bass
import concourse.tile as tile
from concourse import bacc, mybir
from concourse import bass_utils
from concourse._compat import with_exitstack

P = 128
B, S, D = 4, 2048, 1024
NHEAD, DK = 16, 64
HPC = 8      # heads per core
NPAIR = 4    # head pairs per core
DCH = 8      # d_model 128-chunks
NQT = 4      # q tiles of 512
SQT = 512
THETA = 10000.0
SCALE = 0.125          # 1/sqrt(dk)

F32 = mybir.dt.float32
BF16 = mybir.dt.bfloat16
FP8 = mybir.dt.float8e4
PNP = ml_dtypes.bfloat16
P8NP = ml_dtypes.float8_e4m3
WSCALE = 1024.0  # Wq/Wk pre-scale so fp8 values sit in the normal range;
                 # compensated exactly by dividing the cos/sin tables by it
_STATE = None  # compile cache


@with_exitstack
def _attn_kernel(ctx: ExitStack, tc: tile.TileContext, out_ap, ins):
    nc = tc.nc
    xT, xT8, wq, wk, wv, wo, cosF, sinS, ones = ins
    dr = mybir.MatmulPerfMode.DoubleRow

    wpool = ctx.enter_context(tc.tile_pool(name="w", bufs=1))
    xpool = ctx.enter_context(tc.tile_pool(name="x", bufs=2))
    qkpool = ctx.enter_context(tc.tile_pool(name="qk", bufs=1))
    vpool = ctx.enter_context(tc.tile_pool(name="v", bufs=1))
    rpool = ctx.enter_context(tc.tile_pool(name="rope", bufs=4))
    apool = ctx.enter_context(tc.tile_pool(name="attn", bufs=4))
    npool = ctx.enter_context(tc.tile_pool(name="norm", bufs=1))
    rcpool = ctx.enter_context(tc.tile_pool(name="rcp", bufs=2))
    wopool = ctx.enter_context(tc.tile_pool(name="wos", bufs=3))
    # PSUM (8 banks): psS 2x[128,2,512]=4, psO 2x[128,512]=2, psM 2x[128,512]=2
    psS = ctx.enter_context(tc.tile_pool(name="psS", bufs=2, space="PSUM"))
    psO = ctx.enter_context(tc.tile_pool(name="psO", bufs=1, space="PSUM"))
    psM = ctx.enter_context(tc.tile_pool(name="psM", bufs=2, space="PSUM"))

    # ---- resident constants; chunked DMAs ordered for fast ramp-up ----
    cos_sb = wpool.tile([P, S], BF16, tag="cos")
    sin_sb = wpool.tile([P, S], BF16, tag="sin")
    nc.sync.dma_start(cos_sb[:, 0:SQT], cosF[:, 0:SQT])
    nc.sync.dma_start(sin_sb[:, 0:SQT], sinS[:, 0:SQT])
    wq_sb = wpool.tile([P, NPAIR, DCH, P], FP8, tag="wq")
    wk_sb = wpool.tile([P, NPAIR, DCH, P], FP8, tag="wk")
    wv_sb = wpool.tile([P, DCH, HPC * DK], BF16, tag="wv")
    xb80 = xpool.tile([P, DCH, SQT], FP8, tag="x8")
    for c in range(DCH):
        nc.sync.dma_start(xb80[:, c], xT8[:, c, 0:SQT])
    for p in range(NPAIR):
        nc.sync.dma_start(wq_sb[:, p], wq[:, p])
        nc.sync.dma_start(wk_sb[:, p], wk[:, p])
    xb0 = xpool.tile([P, DCH, SQT], BF16, tag="xb")
    for c in range(DCH):
        nc.sync.dma_start(xb0[:, c], xT[:, c, 0:SQT])
        nc.sync.dma_start(wv_sb[:, c], wv[:, c])
    nc.sync.dma_start(cos_sb[:, SQT:], cosF[:, SQT:])
    nc.sync.dma_start(sin_sb[:, SQT:], sinS[:, SQT:])
    wo_sb = wpool.tile([P, NPAIR, D], BF16, tag="wo")

    qk_tiles = {}   # (proj, pair, stile) -> tile [128, 512] bf16
    nrm_tiles = {}  # (pair, qtile) -> tile [128, 512] bf16
    xb_cur, xb8_cur = [None], [None]  # current x tiles for phase_a helpers

    # v tiles created upfront; denominator ones-rows arrive by DMA so no
    # compute engine sits on the critical path for them
    v_tiles = {}    # schunk -> tile [128, 8, 128] bf16
    for sc in range(4 * NQT):
        va = vpool.tile([P, HPC, 2 * DK], BF16, tag=f"v{sc}")
        nc.sync.dma_start(va[:, :, DK:2 * DK], ones)
        v_tiles[sc] = va

    exp_fn = mybir.ActivationFunctionType.Exp

    def qk_group(t, p, proj, w_sb, evac):
        s_sl = slice(t * SQT, (t + 1) * SQT)
        ps = psM.tile([P, SQT], F32, tag="m")
        for c2 in range(0, DCH, 2):
            nc.tensor.matmul(ps[:], w_sb[:, p, c2:c2 + 2],
                             xb8_cur[0][:, c2:c2 + 2],
                             start=(c2 == 0), stop=(c2 == DCH - 2),
                             perf_mode=dr)
        # RoPE: one psum evacuation (alternating engines on the first tile,
        # where nothing else can fill), swap-copies via sbuf-to-sbuf DMA,
        # cos-mul + add on DVE, sin-mul on gpsimd.
        pb = rpool.tile([P, SQT], BF16, tag="pb")
        evac(pb[:], ps[:])
        sw = rpool.tile([P, SQT], BF16, tag="sw")
        for blk, src in ((0, 32), (1, 0), (2, 96), (3, 64)):
            nc.sync.dma_start(sw[32 * blk:32 * blk + 32],
                              pb[src:src + 32])
        u = rpool.tile([P, SQT], BF16, tag="u")
        nc.vector.tensor_mul(u[:], pb[:], cos_sb[:, s_sl])
        w_ = rpool.tile([P, SQT], BF16, tag="wt")
        nc.gpsimd.tensor_mul(w_[:], sw[:], sin_sb[:, s_sl])
        qt = qkpool.tile(
            [P, SQT], BF16,
            tag=f"q{p}_{t % 2}" if proj == "q" else f"k{p}_{t}")
        nc.vector.tensor_add(qt[:], u[:], w_[:])
        qk_tiles[(proj, p, t)] = qt

    def v_group(t, sc4, evac):
        sc = 4 * t + sc4
        ps = psM.tile([P, SQT], F32, tag="m")
        for c in range(DCH):
            nc.tensor.matmul(ps[:], xb_cur[0][:, c, 128 * sc4:128 * sc4 + 128],
                             wv_sb[:, c], start=(c == 0), stop=(c == DCH - 1))
        va = v_tiles[sc]
        evac(va[:, :, 0:DK], ps[:].rearrange("p (h d) -> p h d", d=DK))

    def phase_a(t, xb, xb8):
        xb_cur[0], xb8_cur[0] = xb, xb8
        if t == 0:
            # first tile: nothing overlaps, so spread evacuations over both
            # ACT and DVE and interleave V right after pair 0 to unblock
            # phase_b(0) as early as possible
            ev = [nc.scalar.copy, nc.vector.tensor_copy]
            for proj, w_sb in (("q", wq_sb), ("k", wk_sb)):
                qk_group(t, 0, proj, w_sb, ev[0 if proj == "q" else 1])
            for sc4 in range(4):
                v_group(t, sc4, ev[sc4 % 2])
            for p in range(1, NPAIR):
                for i, (proj, w_sb) in enumerate((("q", wq_sb), ("k", wk_sb))):
                    qk_group(t, p, proj, w_sb, ev[(p + i) % 2])
        else:
            for p in range(NPAIR):
                for proj, w_sb in (("q", wq_sb), ("k", wk_sb)):
                    qk_group(t, p, proj, w_sb, nc.vector.tensor_copy)
            for sc4 in range(4):
                v_group(t, sc4, nc.vector.tensor_copy)

    def phase_b(t):
        nch = 4 * t + 4
        tasks = [(p, kc) for p in range(NPAIR) for kc in range(nch)]
        sts = {}

        def issue_scores(i):
            p, kc = tasks[i]
            delta = max(0, 128 * kc - SQT * t)
            kt = qk_tiles[("k", p, kc // 4)]
            qt = qk_tiles[("q", p, t)]
            ci = kc % 4
            sT = psS.tile([P, 2, SQT], F32, tag="s")
            # two K=64 matmuls issued adjacently into distinct PE row groups
            for h in range(2):
                nc.tensor.matmul(
                    sT[:, h, delta:], kt[64 * h:64 * h + 64, 128 * ci:128 * ci + 128],
                    qt[64 * h:64 * h + 64, delta:], start=True, stop=True,
                    tile_position=(64 * h, 0))
            sts[i] = sT

        issue_scores(0)
        oh = None
        for i, (p, kc) in enumerate(tasks):
            delta = max(0, 128 * kc - SQT * t)
            if kc == 0:
                oh = [psO.tile([P, SQT], F32, tag=f"o{h}", name=f"oh{h}")
                      for h in range(2)]
            sT = sts.pop(i)
            at = apool.tile([P, 2, SQT], BF16, tag="a")
            nc.scalar.activation(at[:, :, delta:], sT[:, :, delta:], exp_fn,
                                 scale=SCALE)
            if 128 * kc >= SQT * t:
                # diagonal block: zero attn where q < k (gpsimd)
                for h in range(2):
                    nc.gpsimd.affine_select(
                        out=at[:, h, delta:delta + 128],
                        in_=at[:, h, delta:delta + 128],
                        compare_op=mybir.AluOpType.is_ge,
                        fill=0.0, base=0,
                        pattern=[[1, 128]], channel_multiplier=-1)
            if i + 1 < len(tasks):
                issue_scores(i + 1)
            va = v_tiles[kc]
            st_, sp_ = (kc == 0), (kc == nch - 1)
            for h in range(2):
                nc.tensor.matmul(
                    oh[h][:, delta:], va[:, 2 * p + h, :],
                    at[:, h, delta:], start=st_, stop=sp_)
            if kc == nch - 1:
                # evacuate outT + denom; psum slot released after these
                # copies. For the final pair (the kernel tail, where ACT has
                # no exps left) split the copies across DVE and ACT.
                last = (t == NQT - 1 and p == NPAIR - 1)
                dncp = nc.scalar.copy if last else nc.vector.tensor_copy
                onrm = npool.tile([P, SQT], BF16, tag=f"n{p}_{t}")
                ohb = rcpool.tile([P, SQT], F32, tag="ohb")
                dnb = rcpool.tile([P, SQT], F32, tag="dnb")
                for h in range(2):
                    nc.vector.tensor_copy(ohb[64 * h:64 * h + 64], oh[h][0:64, :])
                    dncp(dnb[64 * h:64 * h + 64], oh[h][64:128, :])
                rc = rcpool.tile([P, SQT], F32, tag="rc")
                nc.vector.reciprocal_approx_fast(rc[:], dnb[:])
                nc.vector.tensor_mul(onrm[:], ohb[:], rc[:])
                nrm_tiles[(p, t)] = onrm

    def phase_wo(t):
        for qs in range(4):
            for nh in range(2):
                wps = psM.tile([P, SQT], F32, tag="m", name="wps")
                for p in range(NPAIR):
                    nc.tensor.matmul(
                        wps[:], nrm_tiles[(p, t)][:, 128 * qs:128 * qs + 128],
                        wo_sb[:, p, SQT * nh:SQT * (nh + 1)],
                        start=(p == 0), stop=(p == NPAIR - 1))
                # deferred tiles (0..2) run while ACT is exp-bound: keep
                # their evacuations on DVE; the final tile (kernel tail,
                # ACT idle) splits across both.
                st = wopool.tile([P, SQT], F32, tag="wo")
                if t < NQT - 1 or nh == 0:
                    nc.vector.tensor_copy(st[:], wps[:])
                else:
                    nc.scalar.copy(st[:], wps[:])
                nc.sync.dma_start(
                    out_ap[SQT * t + 128 * qs:SQT * t + 128 * qs + 128,
                           SQT * nh:SQT * (nh + 1)], st[:])

    xb, xb8 = xb0, xb80
    for t in range(NQT):
        if t == 0:
            # wo weights: needed first at phase_wo(0); off the ramp-up path
            for pp in range(NPAIR):
                nc.sync.dma_start(wo_sb[:, pp], wo[:, pp])
        phase_a(t, xb, xb8)
        if t + 1 < NQT:
            sl = slice((t + 1) * SQT, (t + 2) * SQT)
            xb8 = xpool.tile([P, DCH, SQT], FP8, tag="x8")
            xb = xpool.tile([P, DCH, SQT], BF16, tag="xb")
            for c in range(DCH):
                nc.sync.dma_start(xb8[:, c], xT8[:, c, sl])
                nc.sync.dma_start(xb[:, c], xT[:, c, sl])
        if t < NQT - 1:
            phase_b(t)
    # Wo projections for tiles 0..2 emitted before the longest attention
    # phase: their matmuls fill tensor-idle gaps while EXP bounds phase_b(3).
    for t in range(NQT - 1):
        phase_wo(t)
    phase_b(NQT - 1)
    phase_wo(NQT - 1)


def _build():
    nc = bacc.Bacc("TRN2", target_bir_lowering=False, debug=False, num_devices=8)
    ins = [
        nc.dram_tensor("xT", [P, DCH, S], BF16, kind="ExternalInput").ap(),
        nc.dram_tensor("xT8", [P, DCH, S], FP8, kind="ExternalInput").ap(),
        nc.dram_tensor("wq", [P, NPAIR, DCH, P], FP8, kind="ExternalInput").ap(),
        nc.dram_tensor("wk", [P, NPAIR, DCH, P], FP8, kind="ExternalInput").ap(),
        nc.dram_tensor("wv", [P, DCH, HPC * DK], BF16, kind="ExternalInput").ap(),
        nc.dram_tensor("wo", [P, NPAIR, D], BF16, kind="ExternalInput").ap(),
        nc.dram_tensor("cosF", [P, S], BF16, kind="ExternalInput").ap(),
        nc.dram_tensor("sinS", [P, S], BF16, kind="ExternalInput").ap(),
        nc.dram_tensor("ones", [P, HPC, DK], BF16, kind="ExternalInput").ap(),
    ]
    out_ap = nc.dram_tensor("out", [S, D], F32, kind="ExternalOutput").ap()
    with tile.TileContext(nc) as tc:
        _attn_kernel(tc, out_ap, ins)
    nc.compile()
    return nc


def _host_prep(x, Wq, Wk, Wv, Wo, token_positions):
    """Build the 8 per-core input maps."""
    x = np.asarray(x, dtype=np.float32)
    Wq = np.asarray(Wq, dtype=np.float32)
    Wk = np.asarray(Wk, dtype=np.float32)
    Wv = np.asarray(Wv, dtype=np.float32)
    Wo = np.asarray(Wo, dtype=np.float32)
    pos = np.asarray(token_positions).astype(np.float64)

    # RoPE tables: rows 0:32 freq-major (even dims), repeated for the 4
    # 32-row blocks; sin signed [-,+,-,+] to implement the rotation swap.
    freqs = 1.0 / (THETA ** (np.arange(0, DK, 2, dtype=np.float64) / DK))  # [32]
    ang = pos[:, None] * freqs[None, :]          # [S, 32]
    cosT = np.cos(ang).T.astype(np.float32)      # [32, S]
    sinT = np.sin(ang).T.astype(np.float32)
    # cos/sin divided by WSCALE**2 overall: the q and k streams each carry a
    # WSCALE factor from the fp8-scaled projection weights; dividing both
    # RoPE tables by WSCALE cancels one factor per stream exactly.
    cosF = np.tile(cosT / WSCALE, (4, 1)).astype(ml_dtypes.bfloat16)
    sinS = (np.concatenate([-sinT, sinT, -sinT, sinT], 0) / WSCALE).astype(
        ml_dtypes.bfloat16)

    xTt = [np.ascontiguousarray(
        x[b].T.reshape(DCH, P, S).transpose(1, 0, 2)) for b in range(B)]
    xTr = [a.astype(PNP) for a in xTt]
    xTr8 = [a.astype(P8NP) for a in xTt]

    def wqk_arr(W, hg):
        perm = np.empty((NPAIR, P), np.int64)
        for p in range(NPAIR):
            hA, hB = 8 * hg + 2 * p, 8 * hg + 2 * p + 1
            perm[p] = np.concatenate([
                DK * hA + np.arange(0, DK, 2), DK * hA + np.arange(1, DK, 2),
                DK * hB + np.arange(0, DK, 2), DK * hB + np.arange(1, DK, 2)])
        a = W[perm] * WSCALE                         # [4, 128, 1024]
        a = a.reshape(NPAIR, P, DCH, P).transpose(3, 0, 2, 1)  # [pi, p, c, m]
        return np.ascontiguousarray(a).astype(P8NP)

    def wv_arr(hg):
        a = Wv[DK * HPC * hg: DK * HPC * (hg + 1)].T   # [1024, 512]
        return np.ascontiguousarray(
            a.reshape(DCH, P, HPC * DK).transpose(1, 0, 2)).astype(PNP)

    def wo_arr(hg):
        a = Wo[:, DK * HPC * hg: DK * HPC * (hg + 1)].T  # [512, 1024]
        return np.ascontiguousarray(
            a.reshape(NPAIR, P, D).transpose(1, 0, 2)).astype(ml_dtypes.bfloat16)

    in_maps = []
    for c in range(8):
        b, hg = c // 2, c % 2
        in_maps.append({
            "xT": xTr[b], "xT8": xTr8[b],
            "wq": wqk_arr(Wq, hg), "wk": wqk_arr(Wk, hg), "wv": wv_arr(hg),
            "wo": wo_arr(hg),
            "cosF": cosF, "sinS": sinS,
            "ones": np.ones((P, HPC, DK), ml_dtypes.bfloat16),
        })
    return in_maps


def prepare(**inputs):
    """Returns (nc, in_maps). Exposed for test.py's traced runs."""
    global _STATE
    if _STATE is None:
        _STATE = _build()
    return _STATE, _host_prep(**inputs)


def kernel(**inputs):
    nc, in_maps = prepare(**inputs)
    res = bass_utils.run_bass_kernel_spmd(nc, in_maps, core_ids=list(range(8)))
    out = np.empty((B, S, D), np.float32)
    for b in range(B):
        out[b] = res.results[2 * b]["out"] + res.results[2 * b + 1]["out"]
    return out
